# revision 13
# baseline (speedup 1.0000x reference)
"""Multi-head causal attention (B=2, S=2048, D=1024, H=16) on 8 NeuronCores.

Sharding: data-parallel over batch (2 groups of 4 cores) x tensor-parallel over
heads (4 heads per core).  Each core projects q/k/v for its 4 heads, runs causal
flash-style attention, normalizes, then the 4 cores of a batch AllGather their
context chunk by chunk ([256,512] per core -> [1024,512]) overlapped with
compute, and each computes a 256-column slice of the output projection.  The
host assembles the 8 output slices.

Precision: projections and output projection run in float32r (TF32-like, 2
PE-cycles/row); the attention core (scores, exp weights, AV) runs in float16
(1 cycle/row, 10-bit mantissa).  Softmax needs no max-subtraction (scaled
scores are bounded, |s| < 4 for this operator family); the denominator comes
from a ones-column appended to V (row 64 of the AV accumulation).
"""
import numpy as np

import concourse.bass as bass
import concourse.mybir as mybir
import concourse.tile as tile
from concourse.bass_utils import run_bass_kernel_spmd

# ---------------------------------------------------------------- constants
B, S, D, H, HD = 2, 2048, 1024, 16, 64
NCORES = 8
HLOC = 4              # heads per core
OLOC = HLOC * HD      # 256 local qkv features
P = 128               # partitions
SBK = 512             # big seq block (moving free dim)
NSB = S // SBK        # 4
NFC = D // P          # 8 feature chunks
NKC = S // P          # 16 key chunks
F32 = mybir.dt.float32
F32R = mybir.dt.float32r
F16 = mybir.dt.float16

_CACHE = {}

# ------------------------------------------------------------- wait legalizer
_wl_counter = [0]


def _legalize_waits(nc):
    """This walrus build allows only ONE inline sync-wait per instruction.
    Move extra waits onto NoOps inserted before, on the same engine stream."""
    for bb in nc.main_func.blocks:
        insts = bb.instructions
        new_list = []
        changed = False
        for inst in insts:
            si = getattr(inst, "sync_info", None)
            waits = list(si.on_wait) if si is not None and si.on_wait else []
            if len(waits) > 1:
                for w in waits[1:]:
                    _wl_counter[0] += 1
                    noop = mybir.InstNoOp(
                        name=f"waitsplit-{_wl_counter[0]}",
                        sync_info=mybir.SyncInfo(on_wait=[w], on_update=[]),
                        bass_nofuse=True,
                        engine=inst.engine,
                    )
                    nc.register_instruction(noop, overwrite=True)
                    new_list.append(noop)
                si.on_wait = waits[:1]
                changed = True
            new_list.append(inst)
        if changed:
            bb.instructions[:] = new_list
    return nc


# ---------------------------------------------------------------- the kernel
def _build_nc():
    nc = bass.Bass(num_devices=NCORES)

    xt = nc.dram_tensor("xt", [D, S], F32R, kind="ExternalInput")
    wq = nc.dram_tensor("wq", [D, OLOC], F32R, kind="ExternalInput")
    wk = nc.dram_tensor("wk", [D, OLOC], F32R, kind="ExternalInput")
    wv = nc.dram_tensor("wv", [D, OLOC], F32R, kind="ExternalInput")
    wo = nc.dram_tensor("wo", [D, OLOC], F32R, kind="ExternalInput")
    bo = nc.dram_tensor("bo", [P, 2], F32, kind="ExternalInput")
    # 4 diagonal masks [128,512] + 4 ones columns for the V augmentation
    masks = nc.dram_tensor("masks", [P, 4 * SBK + 4], F16, kind="ExternalInput")
    ones_in = nc.dram_tensor("ones_in", [1, HD], F32R, kind="ExternalInput")
    outT = nc.dram_tensor("outT", [OLOC, S], F32, kind="ExternalOutput")

    Exp = mybir.ActivationFunctionType.Exp
    Ident = mybir.ActivationFunctionType.Identity

    with tile.TileContext(nc) as tc:
        with (
            tc.tile_pool(name="const", bufs=1) as constp,
            tc.tile_pool(name="wsb", bufs=1) as wsb,
            tc.tile_pool(name="qk", bufs=1) as qkp,
            tc.tile_pool(name="vtp", bufs=1) as vtp,
            tc.tile_pool(name="ctx", bufs=1) as ctxp_pool,
            tc.tile_pool(name="xc", bufs=10) as xcp,
            tc.tile_pool(name="exp", bufs=6) as expp,
            tc.tile_pool(name="sm", bufs=2) as smp,
            tc.tile_pool(name="gth", bufs=1) as gthp,
            tc.tile_pool(name="osb", bufs=3) as osbp,
            tc.tile_pool(name="dram", bufs=1, space="DRAM") as dramp,
            tc.tile_pool(name="psa", bufs=5, space="PSUM") as psa,
            tc.tile_pool(name="psC", bufs=2, space="PSUM") as psC,
            tc.tile_pool(name="psB", bufs=1, space="PSUM") as psB,
        ):
            # ---- weights, interleaved by feature-chunk for a fast start
            wq_sb = wsb.tile([P, NFC * OLOC], F32R, tag="wq")
            wk_sb = wsb.tile([P, NFC * OLOC], F32R, tag="wk")
            wv_sb = wsb.tile([P, NFC * OLOC], F32R, tag="wv")
            wo_sb = constp.tile([P, NFC * OLOC], F32R)
            for fc in range(NFC):
                for t_sb, t_dr in ((wq_sb, wq), (wk_sb, wk), (wv_sb, wv)):
                    nc.sync.dma_start(
                        t_sb[:, fc * OLOC:(fc + 1) * OLOC],
                        t_dr[fc * P:(fc + 1) * P, :],
                    )

            # ---- persistent activations (fp16 attention operands)
            qT = [[qkp.tile([P, SBK], F16, name=f"qT{ob}{sb}", tag=f"q{ob}{sb}")
                   for sb in range(NSB)] for ob in range(2)]
            kT = [[qkp.tile([P, SBK], F16, name=f"kT{ob}{sb}", tag=f"k{ob}{sb}")
                   for sb in range(NSB)] for ob in range(2)]
            # v natural per key-chunk: [128, 4*65]; head h at cols 65h..65h+63,
            # ones column at 65h+64 (softmax denominator lands in AV row 64).
            vt = [vtp.tile([P, HLOC * (HD + 1)], F16, name=f"vt{kc}", tag=f"v{kc}")
                  for kc in range(NKC)]
            ctxq = [[ctxp_pool.tile([P, SBK], F32R, name=f"ctxq{ob}{qb}",
                                    tag=f"c{ob}{qb}") for qb in range(NSB)]
                    for ob in range(2)]

            masks_sb = constp.tile([P, 4 * SBK + 4], F16)
            nc.sync.dma_start(masks_sb[:], masks[:])
            bo_sb = constp.tile([P, 2], F32)
            nc.sync.dma_start(bo_sb[:], bo[:])
            ones_l = constp.tile([1, HD], F32R)
            nc.sync.dma_start(ones_l[:], ones_in[:])
            for kc in range(NKC):
                v3 = vt[kc].rearrange("p (h x) -> p h x", h=HLOC)
                nc.sync.dma_start(v3[:, :, HD:HD + 1], masks_sb[:, 4 * SBK:])
            for fc in range(NFC):
                nc.sync.dma_start(
                    wo_sb[:, fc * OLOC:(fc + 1) * OLOC], wo[fc * P:(fc + 1) * P, :])

            cin_q = [dramp.tile([OLOC, SBK], F32R, name=f"cin{qb}", tag=f"cin{qb}")
                     for qb in range(NSB)]
            cout_q = [dramp.tile([HLOC * OLOC, SBK], F32R, name=f"cout{qb}",
                                 tag=f"cout{qb}") for qb in range(NSB)]

            # ======== one fused pass per seq-block: proj -> attn -> AG -> out
            for sb in range(NSB):
                # ---- q/k/v projections for this seq block
                xch = []
                for fc in range(NFC):
                    t = xcp.tile([P, SBK], F32R, tag="x", name=f"x{sb}{fc}")
                    nc.sync.dma_start(
                        t[:], xt[fc * P:(fc + 1) * P, sb * SBK:(sb + 1) * SBK])
                    xch.append(t)
                for (w_sb, dst) in ((wq_sb, qT), (wk_sb, kT)):
                    for ob in range(2):
                        ps = psa.tile([P, SBK], F32, tag="acc", name=f"pq{sb}{ob}")
                        for fc in range(NFC):
                            lhsT = w_sb[:, fc * OLOC + ob * P:
                                        fc * OLOC + (ob + 1) * P]
                            nc.tensor.matmul(ps[:], lhsT, xch[fc][:],
                                             start=(fc == 0), stop=(fc == NFC - 1))
                        nc.vector.tensor_copy(dst[ob][sb][:], ps[:])
                for sc in range(4):
                    kc = 4 * sb + sc
                    ps = psa.tile([P, OLOC], F32, tag="acc", name=f"pv{sb}{sc}")
                    for fc in range(NFC):
                        lhsT = xch[fc][:, sc * P:(sc + 1) * P]
                        nc.tensor.matmul(ps[:], lhsT,
                                         wv_sb[:, fc * OLOC:(fc + 1) * OLOC],
                                         start=(fc == 0), stop=(fc == NFC - 1))
                    v3 = vt[kc].rearrange("p (h x) -> p h x", h=HLOC)
                    p3 = ps.rearrange("p (h x) -> p h x", h=HLOC)
                    nc.vector.tensor_copy(v3[:, :, 0:HD], p3[:])

                # ---- causal attention for q-chunk qb == sb
                qb = sb
                nkb = 4 * qb + 4
                # head h's denominator parked at partition 32h (alignment)
                den = smp.tile([P, SBK], F32, name=f"den{qb}", tag="den")
                rec = smp.tile([P, SBK], F32, name=f"rec{qb}", tag="rec")
                ctx_sb_h = []
                for ob in range(2):
                    for j in range(2):
                        h = 2 * ob + j
                        ctx_ps = psC.tile([HD + 1, SBK], F32, tag="ctx")
                        for kb in range(nkb):
                            t = kb - 4 * qb
                            off = 128 * t if t > 0 else 0
                            sp = psa.tile([P, SBK], F32, tag="acc",
                                          name=f"sp{qb}{h}{kb}")
                            nc.tensor.matmul(
                                sp[:, off:],
                                kT[ob][kb // 4][j * HD:(j + 1) * HD,
                                                (kb % 4) * P:(kb % 4 + 1) * P],
                                qT[ob][qb][j * HD:(j + 1) * HD, off:],
                                start=True, stop=True,
                                tile_position=(j * HD, 0),
                            )
                            et = expp.tile([P, SBK], F16, tag="e",
                                           name=f"et{qb}{h}{kb}")
                            nc.scalar.activation(et[:, off:], sp[:, off:],
                                                 Exp, scale=0.125)
                            if t >= 0:  # diagonal block: zero out k > q
                                nc.vector.tensor_tensor(
                                    et[:, off:], et[:, off:],
                                    masks_sb[:, t * SBK + off:(t + 1) * SBK],
                                    mybir.AluOpType.mult,
                                )
                            nc.tensor.matmul(
                                ctx_ps[:, off:],
                                vt[kb][:, h * (HD + 1):(h + 1) * (HD + 1)],
                                et[:, off:],
                                start=(kb == 0), stop=(kb == nkb - 1),
                            )
                        nc.vector.tensor_copy(den[32 * h:32 * h + 1, :],
                                              ctx_ps[HD:HD + 1, :])
                        cu = smp.tile([HD, SBK], F32, name=f"cu{qb}{h}",
                                      tag=f"cu{h}")
                        nc.scalar.copy(cu[:], ctx_ps[0:HD, :])
                        ctx_sb_h.append(cu)
                # batched reciprocal for the 4 heads of this q-chunk
                nc.vector.reciprocal(rec[:], den[:])
                for ob in range(2):
                    for j in range(2):
                        h = 2 * ob + j
                        rcr = smp.tile([1, SBK], F32R, tag="rcr",
                                       name=f"rcr{qb}{h}")
                        nc.vector.tensor_copy(rcr[:], rec[32 * h:32 * h + 1, :])
                        bc_ps = psB.tile([HD, SBK], F32, tag="bcp",
                                         name=f"bcp{qb}{h}")
                        nc.tensor.matmul(bc_ps[:], ones_l[:], rcr[:],
                                         start=True, stop=True)
                        bc = smp.tile([HD, SBK], F32, tag="bc", name=f"bc{qb}{h}")
                        nc.vector.tensor_copy(bc[:], bc_ps[:])
                        nc.vector.tensor_tensor(
                            ctxq[ob][qb][j * HD:(j + 1) * HD, :],
                            ctx_sb_h[h][:], bc[:],
                            mybir.AluOpType.mult,
                        )
                # ---- AllGather this q-chunk across the 4 cores of the batch
                nc.gpsimd.dma_start(cin_q[qb][0:P, :], ctxq[0][qb][:])
                nc.gpsimd.dma_start(cin_q[qb][P:OLOC, :], ctxq[1][qb][:])
                nc.gpsimd.collective_compute(
                    "AllGather",
                    mybir.AluOpType.bypass,
                    replica_groups=[[0, 1, 2, 3], [4, 5, 6, 7]],
                    ins=[cin_q[qb].opt()],
                    outs=[cout_q[qb].opt()],
                )
                g = [gthp.tile([P, SBK], F32R, name=f"g{qb}{oc}", tag=f"g{oc}")
                     for oc in range(NFC)]
                for oc in range(NFC):
                    nc.gpsimd.dma_start(g[oc][:],
                                        cout_q[qb][oc * P:(oc + 1) * P, :])
                for cb in range(2):
                    ps = psa.tile([P, SBK], F32, tag="acc", name=f"po{qb}{cb}")
                    for oc in range(NFC):
                        lhsT = wo_sb[:, oc * OLOC + cb * P:
                                     oc * OLOC + (cb + 1) * P]
                        nc.tensor.matmul(ps[:], lhsT, g[oc][:],
                                         start=(oc == 0), stop=(oc == NFC - 1))
                    ot = osbp.tile([P, SBK], F32, tag="ot", name=f"ot{qb}{cb}")
                    nc.scalar.activation(ot[:], ps[:], Ident,
                                         bias=bo_sb[:, cb:cb + 1], scale=1.0)
                    nc.sync.dma_start(
                        outT[cb * P:(cb + 1) * P, qb * SBK:(qb + 1) * SBK],
                        ot[:])

    _legalize_waits(nc)
    return nc


def _get_nc():
    if "nc" not in _CACHE:
        _CACHE["nc"] = _build_nc()
    return _CACHE["nc"]


LAST_RESULTS = None  # BassKernelResults of the most recent run (for profiling)


def kernel(x, Wq, Wk, Wv, Wo, bo):
    global LAST_RESULTS
    x = np.ascontiguousarray(np.asarray(x, dtype=np.float32))
    Wq = np.asarray(Wq, dtype=np.float32)
    Wk = np.asarray(Wk, dtype=np.float32)
    Wv = np.asarray(Wv, dtype=np.float32)
    Wo = np.asarray(Wo, dtype=np.float32)
    bo = np.asarray(bo, dtype=np.float32)

    # causal masks for the 4 diagonal 128x512 blocks (valid iff qi >= ki+128t)
    # plus 4 trailing ones-columns used to augment V.
    ki = np.arange(P)[:, None]
    qi = np.arange(SBK)[None, :]
    masks = np.concatenate(
        [(qi >= ki + P * t).astype(np.float16) for t in range(4)]
        + [np.ones((P, 4), dtype=np.float16)], axis=1)

    in_maps = []
    for c in range(NCORES):
        b, g = divmod(c, HLOC)
        sl = slice(g * OLOC, (g + 1) * OLOC)
        in_maps.append({
            "xt": np.ascontiguousarray(x[b].T),
            "wq": np.ascontiguousarray(Wq[sl, :].T),
            "wk": np.ascontiguousarray(Wk[sl, :].T),
            "wv": np.ascontiguousarray(Wv[sl, :].T),
            "wo": np.ascontiguousarray(Wo[sl, :].T),
            "bo": np.ascontiguousarray(bo[sl].reshape(2, P).T),
            "masks": masks,
            "ones_in": np.ones((1, HD), dtype=np.float32),
        })

    nc = _get_nc()
    LAST_RESULTS = run_bass_kernel_spmd(nc, in_maps, core_ids=list(range(NCORES)))

    out = np.empty((B, S, D), dtype=np.float32)
    for c in range(NCORES):
        b, g = divmod(c, HLOC)
        out[b, :, g * OLOC:(g + 1) * OLOC] = LAST_RESULTS.results[c]["outT"].T
    return out


# revision 14
# speedup vs baseline: 1.4314x; 1.4314x over previous
"""Multi-head causal attention (B=2, S=2048, D=1024, H=16) on 8 NeuronCores.

Sharding: data-parallel over batch (2 groups of 4 cores) x tensor-parallel over
heads (4 heads per core).  Each core projects q/k/v for its 4 heads, runs causal
flash-style attention, normalizes, then the 4 cores of a batch AllGather their
context chunk by chunk ([256,512] per core -> [1024,512]) overlapped with
compute, and each computes a 256-column slice of the output projection.  The
host assembles the 8 output slices.

Precision: projections and output projection run in float32r (TF32-like, 2
PE-cycles/row); the attention core (scores, exp weights, AV) runs in float16
(1 cycle/row, 10-bit mantissa).  Softmax needs no max-subtraction (scaled
scores are bounded, |s| < 4 for this operator family); the denominator comes
from a ones-column appended to V (row 64 of the AV accumulation).
"""
import numpy as np

import concourse.bass as bass
import concourse.mybir as mybir
import concourse.tile as tile
from concourse.bass_utils import run_bass_kernel_spmd

# ---------------------------------------------------------------- constants
B, S, D, H, HD = 2, 2048, 1024, 16, 64
NCORES = 8
HLOC = 4              # heads per core
OLOC = HLOC * HD      # 256 local qkv features
P = 128               # partitions
SBK = 512             # big seq block (moving free dim)
NSB = S // SBK        # 4
NFC = D // P          # 8 feature chunks
NKC = S // P          # 16 key chunks
F32 = mybir.dt.float32
F32R = mybir.dt.float32r
F16 = mybir.dt.float16

_CACHE = {}

# ------------------------------------------------------------- wait legalizer
_wl_counter = [0]


def _legalize_waits(nc):
    """This walrus build allows only ONE inline sync-wait per instruction.
    Move extra waits onto NoOps inserted before, on the same engine stream."""
    for bb in nc.main_func.blocks:
        insts = bb.instructions
        new_list = []
        changed = False
        for inst in insts:
            si = getattr(inst, "sync_info", None)
            waits = list(si.on_wait) if si is not None and si.on_wait else []
            if len(waits) > 1:
                for w in waits[1:]:
                    _wl_counter[0] += 1
                    noop = mybir.InstNoOp(
                        name=f"waitsplit-{_wl_counter[0]}",
                        sync_info=mybir.SyncInfo(on_wait=[w], on_update=[]),
                        bass_nofuse=True,
                        engine=inst.engine,
                    )
                    nc.register_instruction(noop, overwrite=True)
                    new_list.append(noop)
                si.on_wait = waits[:1]
                changed = True
            new_list.append(inst)
        if changed:
            bb.instructions[:] = new_list
    return nc


# ---------------------------------------------------------------- the kernel
def _build_nc():
    nc = bass.Bass(num_devices=NCORES)

    xt = nc.dram_tensor("xt", [D, S], F32R, kind="ExternalInput")
    wq = nc.dram_tensor("wq", [D, OLOC], F32R, kind="ExternalInput")
    wk = nc.dram_tensor("wk", [D, OLOC], F32R, kind="ExternalInput")
    wv = nc.dram_tensor("wv", [D, OLOC], F32R, kind="ExternalInput")
    wo = nc.dram_tensor("wo", [D, OLOC], F32R, kind="ExternalInput")
    bo = nc.dram_tensor("bo", [P, 2], F32, kind="ExternalInput")
    # 4 diagonal masks [128,512] + 4 ones columns for the V augmentation
    masks = nc.dram_tensor("masks", [P, 4 * SBK + 4], F16, kind="ExternalInput")
    ones_in = nc.dram_tensor("ones_in", [1, HD], F32R, kind="ExternalInput")
    outT = nc.dram_tensor("outT", [OLOC, S], F32, kind="ExternalOutput")

    Exp = mybir.ActivationFunctionType.Exp
    Ident = mybir.ActivationFunctionType.Identity

    with tile.TileContext(nc) as tc:
        with (
            tc.tile_pool(name="const", bufs=1) as constp,
            tc.tile_pool(name="wsb", bufs=1) as wsb,
            tc.tile_pool(name="qk", bufs=1) as qkp,
            tc.tile_pool(name="vtp", bufs=1) as vtp,
            tc.tile_pool(name="ctx", bufs=1) as ctxp_pool,
            tc.tile_pool(name="xc", bufs=10) as xcp,
            tc.tile_pool(name="exp", bufs=6) as expp,
            tc.tile_pool(name="sm", bufs=2) as smp,
            tc.tile_pool(name="gth", bufs=1) as gthp,
            tc.tile_pool(name="osb", bufs=3) as osbp,
            tc.tile_pool(name="dram", bufs=1, space="DRAM") as dramp,
            tc.tile_pool(name="psa", bufs=5, space="PSUM") as psa,
            tc.tile_pool(name="psC", bufs=2, space="PSUM") as psC,
            tc.tile_pool(name="psB", bufs=1, space="PSUM") as psB,
        ):
            # ---- weights, interleaved by feature-chunk for a fast start
            wq_sb = wsb.tile([P, NFC * OLOC], F32R, tag="wq")
            wk_sb = wsb.tile([P, NFC * OLOC], F32R, tag="wk")
            wv_sb = wsb.tile([P, NFC * OLOC], F32R, tag="wv")
            wo_sb = constp.tile([P, NFC * OLOC], F32R)
            xch0 = []
            for fc in range(NFC):
                for t_sb, t_dr in ((wq_sb, wq), (wk_sb, wk)):
                    nc.sync.dma_start(
                        t_sb[:, fc * OLOC:(fc + 1) * OLOC],
                        t_dr[fc * P:(fc + 1) * P, :],
                    )
                t = xcp.tile([P, SBK], F32R, tag="x", name=f"x0{fc}")
                nc.sync.dma_start(t[:], xt[fc * P:(fc + 1) * P, 0:SBK])
                xch0.append(t)
            for fc in range(NFC):
                nc.sync.dma_start(
                    wv_sb[:, fc * OLOC:(fc + 1) * OLOC], wv[fc * P:(fc + 1) * P, :])

            # ---- persistent activations (fp16 attention operands)
            qT = [[qkp.tile([P, SBK], F16, name=f"qT{ob}{sb}", tag=f"q{ob}{sb}")
                   for sb in range(NSB)] for ob in range(2)]
            kT = [[qkp.tile([P, SBK], F16, name=f"kT{ob}{sb}", tag=f"k{ob}{sb}")
                   for sb in range(NSB)] for ob in range(2)]
            # v natural per key-chunk: [128, 4*65]; head h at cols 65h..65h+63,
            # ones column at 65h+64 (softmax denominator lands in AV row 64).
            vt = [vtp.tile([P, HLOC * (HD + 1)], F16, name=f"vt{kc}", tag=f"v{kc}")
                  for kc in range(NKC)]
            ctxq = [[ctxp_pool.tile([P, SBK], F32R, name=f"ctxq{ob}{qb}",
                                    tag=f"c{ob}{qb}") for qb in range(NSB)]
                    for ob in range(2)]

            masks_sb = constp.tile([P, 4 * SBK + 4], F16)
            nc.sync.dma_start(masks_sb[:], masks[:])
            bo_sb = constp.tile([P, 2], F32)
            nc.sync.dma_start(bo_sb[:], bo[:])
            ones_l = constp.tile([1, HD], F32R)
            nc.sync.dma_start(ones_l[:], ones_in[:])
            for kc in range(NKC):
                v3 = vt[kc].rearrange("p (h x) -> p h x", h=HLOC)
                nc.gpsimd.dma_start(v3[:, :, HD:HD + 1], masks_sb[:, 4 * SBK:])
            for fc in range(NFC):
                nc.sync.dma_start(
                    wo_sb[:, fc * OLOC:(fc + 1) * OLOC], wo[fc * P:(fc + 1) * P, :])

            cin_q = [dramp.tile([OLOC, SBK], F32R, name=f"cin{qb}", tag=f"cin{qb}")
                     for qb in range(NSB)]
            cout_q = [dramp.tile([HLOC * OLOC, SBK], F32R, name=f"cout{qb}",
                                 tag=f"cout{qb}") for qb in range(NSB)]

            # ======== one fused pass per seq-block: proj -> attn -> AG;
            # the out-projection of chunk qb is emitted one block later so the
            # PE never waits on the AllGather.
            pending_outproj = []
            for sb in range(NSB):
                # ---- q/k/v projections for this seq block
                if sb == 0:
                    xch = xch0
                else:
                    xch = []
                    for fc in range(NFC):
                        t = xcp.tile([P, SBK], F32R, tag="x", name=f"x{sb}{fc}")
                        nc.sync.dma_start(
                            t[:], xt[fc * P:(fc + 1) * P, sb * SBK:(sb + 1) * SBK])
                        xch.append(t)
                for (w_sb, dst) in ((wq_sb, qT), (wk_sb, kT)):
                    for ob in range(2):
                        ps = psa.tile([P, SBK], F32, tag="acc", name=f"pq{sb}{ob}")
                        for fc in range(NFC):
                            lhsT = w_sb[:, fc * OLOC + ob * P:
                                        fc * OLOC + (ob + 1) * P]
                            nc.tensor.matmul(ps[:], lhsT, xch[fc][:],
                                             start=(fc == 0), stop=(fc == NFC - 1))
                        nc.vector.tensor_copy(dst[ob][sb][:], ps[:])
                for sc in range(4):
                    kc = 4 * sb + sc
                    ps = psa.tile([P, OLOC], F32, tag="acc", name=f"pv{sb}{sc}")
                    for fc in range(NFC):
                        lhsT = xch[fc][:, sc * P:(sc + 1) * P]
                        nc.tensor.matmul(ps[:], lhsT,
                                         wv_sb[:, fc * OLOC:(fc + 1) * OLOC],
                                         start=(fc == 0), stop=(fc == NFC - 1))
                    v3 = vt[kc].rearrange("p (h x) -> p h x", h=HLOC)
                    p3 = ps.rearrange("p (h x) -> p h x", h=HLOC)
                    nc.vector.tensor_copy(v3[:, :, 0:HD], p3[:])

                # ---- causal attention for q-chunk qb == sb
                qb = sb
                nkb = 4 * qb + 4
                # head h's denominator parked at partition 32h (alignment)
                den = smp.tile([P, SBK], F32, name=f"den{qb}", tag="den")
                rec = smp.tile([P, SBK], F32, name=f"rec{qb}", tag="rec")
                ctx_sb_h = []
                for ob in range(2):
                    for j in range(2):
                        h = 2 * ob + j
                        ctx_ps = psC.tile([HD + 1, SBK], F32, tag="ctx")
                        for kb in range(nkb):
                            t = kb - 4 * qb
                            off = 128 * t if t > 0 else 0
                            sp = psa.tile([P, SBK], F32, tag="acc",
                                          name=f"sp{qb}{h}{kb}")
                            nc.tensor.matmul(
                                sp[:, off:],
                                kT[ob][kb // 4][j * HD:(j + 1) * HD,
                                                (kb % 4) * P:(kb % 4 + 1) * P],
                                qT[ob][qb][j * HD:(j + 1) * HD, off:],
                                start=True, stop=True,
                                tile_position=(j * HD, 0),
                            )
                            et = expp.tile([P, SBK], F16, tag="e",
                                           name=f"et{qb}{h}{kb}")
                            nc.scalar.activation(et[:, off:], sp[:, off:],
                                                 Exp, scale=0.125)
                            if t >= 0:  # diagonal block: zero out k > q
                                nc.vector.tensor_tensor(
                                    et[:, off:], et[:, off:],
                                    masks_sb[:, t * SBK + off:(t + 1) * SBK],
                                    mybir.AluOpType.mult,
                                )
                            nc.tensor.matmul(
                                ctx_ps[:, off:],
                                vt[kb][:, h * (HD + 1):(h + 1) * (HD + 1)],
                                et[:, off:],
                                start=(kb == 0), stop=(kb == nkb - 1),
                            )
                        nc.vector.tensor_copy(den[32 * h:32 * h + 1, :],
                                              ctx_ps[HD:HD + 1, :])
                        cu = smp.tile([HD, SBK], F32, name=f"cu{qb}{h}",
                                      tag=f"cu{h}")
                        nc.scalar.copy(cu[:], ctx_ps[0:HD, :])
                        ctx_sb_h.append(cu)
                # batched reciprocal for the 4 heads of this q-chunk
                nc.vector.reciprocal(rec[:], den[:])
                for ob in range(2):
                    for j in range(2):
                        h = 2 * ob + j
                        rcr = smp.tile([1, SBK], F32R, tag="rcr",
                                       name=f"rcr{qb}{h}")
                        nc.vector.tensor_copy(rcr[:], rec[32 * h:32 * h + 1, :])
                        bc_ps = psB.tile([HD, SBK], F32, tag="bcp",
                                         name=f"bcp{qb}{h}")
                        nc.tensor.matmul(bc_ps[:], ones_l[:], rcr[:],
                                         start=True, stop=True)
                        bc = smp.tile([HD, SBK], F32, tag="bc", name=f"bc{qb}{h}")
                        nc.vector.tensor_copy(bc[:], bc_ps[:])
                        nc.vector.tensor_tensor(
                            ctxq[ob][qb][j * HD:(j + 1) * HD, :],
                            ctx_sb_h[h][:], bc[:],
                            mybir.AluOpType.mult,
                        )
                # ---- AllGather this q-chunk across the 4 cores of the batch
                nc.gpsimd.dma_start(cin_q[qb][0:P, :], ctxq[0][qb][:])
                nc.gpsimd.dma_start(cin_q[qb][P:OLOC, :], ctxq[1][qb][:])
                nc.gpsimd.collective_compute(
                    "AllGather",
                    mybir.AluOpType.bypass,
                    replica_groups=[[0, 1, 2, 3], [4, 5, 6, 7]],
                    ins=[cin_q[qb].opt()],
                    outs=[cout_q[qb].opt()],
                )
                def _emit_outproj(qb=qb):
                    g = [gthp.tile([P, SBK], F32R, name=f"g{qb}{oc}",
                                   tag=f"g{qb % 2}{oc}") for oc in range(NFC)]
                    for oc in range(NFC):
                        nc.gpsimd.dma_start(g[oc][:],
                                            cout_q[qb][oc * P:(oc + 1) * P, :])
                    for cb in range(2):
                        ps = psa.tile([P, SBK], F32, tag="acc",
                                      name=f"po{qb}{cb}")
                        for oc in range(NFC):
                            lhsT = wo_sb[:, oc * OLOC + cb * P:
                                         oc * OLOC + (cb + 1) * P]
                            nc.tensor.matmul(ps[:], lhsT, g[oc][:],
                                             start=(oc == 0),
                                             stop=(oc == NFC - 1))
                        ot = osbp.tile([P, SBK], F32, tag="ot",
                                       name=f"ot{qb}{cb}")
                        nc.scalar.activation(ot[:], ps[:], Ident,
                                             bias=bo_sb[:, cb:cb + 1], scale=1.0)
                        nc.sync.dma_start(
                            outT[cb * P:(cb + 1) * P,
                                 qb * SBK:(qb + 1) * SBK], ot[:])
                pending_outproj.append(_emit_outproj)
                if sb >= 1:
                    pending_outproj.pop(0)()
            while pending_outproj:
                pending_outproj.pop(0)()

    _legalize_waits(nc)
    return nc


def _get_nc():
    if "nc" not in _CACHE:
        _CACHE["nc"] = _build_nc()
    return _CACHE["nc"]


LAST_RESULTS = None  # BassKernelResults of the most recent run (for profiling)


def kernel(x, Wq, Wk, Wv, Wo, bo):
    global LAST_RESULTS
    x = np.ascontiguousarray(np.asarray(x, dtype=np.float32))
    Wq = np.asarray(Wq, dtype=np.float32)
    Wk = np.asarray(Wk, dtype=np.float32)
    Wv = np.asarray(Wv, dtype=np.float32)
    Wo = np.asarray(Wo, dtype=np.float32)
    bo = np.asarray(bo, dtype=np.float32)

    # causal masks for the 4 diagonal 128x512 blocks (valid iff qi >= ki+128t)
    # plus 4 trailing ones-columns used to augment V.
    ki = np.arange(P)[:, None]
    qi = np.arange(SBK)[None, :]
    masks = np.concatenate(
        [(qi >= ki + P * t).astype(np.float16) for t in range(4)]
        + [np.ones((P, 4), dtype=np.float16)], axis=1)

    in_maps = []
    for c in range(NCORES):
        b, g = divmod(c, HLOC)
        sl = slice(g * OLOC, (g + 1) * OLOC)
        in_maps.append({
            "xt": np.ascontiguousarray(x[b].T),
            "wq": np.ascontiguousarray(Wq[sl, :].T),
            "wk": np.ascontiguousarray(Wk[sl, :].T),
            "wv": np.ascontiguousarray(Wv[sl, :].T),
            "wo": np.ascontiguousarray(Wo[sl, :].T),
            "bo": np.ascontiguousarray(bo[sl].reshape(2, P).T),
            "masks": masks,
            "ones_in": np.ones((1, HD), dtype=np.float32),
        })

    nc = _get_nc()
    LAST_RESULTS = run_bass_kernel_spmd(nc, in_maps, core_ids=list(range(NCORES)))

    out = np.empty((B, S, D), dtype=np.float32)
    for c in range(NCORES):
        b, g = divmod(c, HLOC)
        out[b, :, g * OLOC:(g + 1) * OLOC] = LAST_RESULTS.results[c]["outT"].T
    return out


# revision 15
# speedup vs baseline: 1.6884x; 1.1795x over previous
"""Multi-head causal attention (B=2, S=2048, D=1024, H=16) on 8 NeuronCores.

Sharding: data-parallel over batch (2 groups of 4 cores) x tensor-parallel over
heads (4 heads per core).  Each core projects q/k/v for its 4 heads, runs causal
flash-style attention, normalizes, then the 4 cores of a batch AllGather their
context chunk by chunk ([256,512] per core -> [1024,512]) overlapped with
compute, and each computes a 256-column slice of the output projection.  The
host assembles the 8 output slices.

Precision: projections and output projection run in float32r (TF32-like, 2
PE-cycles/row); the attention core (scores, exp weights, AV) runs in float16
(1 cycle/row, 10-bit mantissa).  Softmax needs no max-subtraction (scaled
scores are bounded, |s| < 4 for this operator family); the denominator comes
from a ones-column appended to V (row 64 of the AV accumulation).
"""
import numpy as np

import concourse.bass as bass
import concourse.mybir as mybir
import concourse.tile as tile
from concourse.bass_utils import run_bass_kernel_spmd

# ---------------------------------------------------------------- constants
B, S, D, H, HD = 2, 2048, 1024, 16, 64
NCORES = 8
HLOC = 4              # heads per core
OLOC = HLOC * HD      # 256 local qkv features
P = 128               # partitions
SBK = 512             # big seq block (moving free dim)
NSB = S // SBK        # 4
NFC = D // P          # 8 feature chunks
NKC = S // P          # 16 key chunks
F32 = mybir.dt.float32
F32R = mybir.dt.float32r
F16 = mybir.dt.float16

_CACHE = {}

# ------------------------------------------------------------- wait legalizer
_wl_counter = [0]


def _legalize_waits(nc):
    """This walrus build allows only ONE inline sync-wait per instruction.
    Move extra waits onto NoOps inserted before, on the same engine stream."""
    for bb in nc.main_func.blocks:
        insts = bb.instructions
        new_list = []
        changed = False
        for inst in insts:
            si = getattr(inst, "sync_info", None)
            waits = list(si.on_wait) if si is not None and si.on_wait else []
            if len(waits) > 1:
                for w in waits[1:]:
                    _wl_counter[0] += 1
                    noop = mybir.InstNoOp(
                        name=f"waitsplit-{_wl_counter[0]}",
                        sync_info=mybir.SyncInfo(on_wait=[w], on_update=[]),
                        bass_nofuse=True,
                        engine=inst.engine,
                    )
                    nc.register_instruction(noop, overwrite=True)
                    new_list.append(noop)
                si.on_wait = waits[:1]
                changed = True
            new_list.append(inst)
        if changed:
            bb.instructions[:] = new_list
    return nc


# ---------------------------------------------------------------- the kernel
def _build_nc():
    nc = bass.Bass(num_devices=NCORES)

    xt = nc.dram_tensor("xt", [D, S], F16, kind="ExternalInput")
    wq = nc.dram_tensor("wq", [D, OLOC], F16, kind="ExternalInput")
    wk = nc.dram_tensor("wk", [D, OLOC], F16, kind="ExternalInput")
    wv = nc.dram_tensor("wv", [D, OLOC], F16, kind="ExternalInput")
    wo = nc.dram_tensor("wo", [D, OLOC], F16, kind="ExternalInput")
    bo = nc.dram_tensor("bo", [P, 2], F32, kind="ExternalInput")
    # 4 diagonal masks [128,512] + 4 ones columns for the V augmentation
    masks = nc.dram_tensor("masks", [P, 4 * SBK + 4], F16, kind="ExternalInput")
    ones_in = nc.dram_tensor("ones_in", [1, HD], F32R, kind="ExternalInput")
    outT = nc.dram_tensor("outT", [OLOC, S], F32, kind="ExternalOutput")

    Exp = mybir.ActivationFunctionType.Exp
    Ident = mybir.ActivationFunctionType.Identity

    with tile.TileContext(nc) as tc:
        with (
            tc.tile_pool(name="const", bufs=1) as constp,
            tc.tile_pool(name="wsb", bufs=1) as wsb,
            tc.tile_pool(name="qk", bufs=1) as qkp,
            tc.tile_pool(name="vtp", bufs=1) as vtp,
            tc.tile_pool(name="ctx", bufs=1) as ctxp_pool,
            tc.tile_pool(name="xc", bufs=10) as xcp,
            tc.tile_pool(name="exp", bufs=6) as expp,
            tc.tile_pool(name="sm", bufs=2) as smp,
            tc.tile_pool(name="gth", bufs=1) as gthp,
            tc.tile_pool(name="osb", bufs=3) as osbp,
            tc.tile_pool(name="dram", bufs=1, space="DRAM") as dramp,
            tc.tile_pool(name="psa", bufs=5, space="PSUM") as psa,
            tc.tile_pool(name="psC", bufs=2, space="PSUM") as psC,
            tc.tile_pool(name="psB", bufs=1, space="PSUM") as psB,
        ):
            # ---- weights, interleaved by feature-chunk for a fast start
            wq_sb = wsb.tile([P, NFC * OLOC], F16, tag="wq")
            wk_sb = wsb.tile([P, NFC * OLOC], F16, tag="wk")
            wv_sb = wsb.tile([P, NFC * OLOC], F16, tag="wv")
            wo_sb = constp.tile([P, NFC * OLOC], F16)
            xch0 = []
            for fc in range(NFC):
                for t_sb, t_dr in ((wq_sb, wq), (wk_sb, wk)):
                    nc.sync.dma_start(
                        t_sb[:, fc * OLOC:(fc + 1) * OLOC],
                        t_dr[fc * P:(fc + 1) * P, :],
                    )
                t = xcp.tile([P, SBK], F16, tag="x", name=f"x0{fc}")
                nc.sync.dma_start(t[:], xt[fc * P:(fc + 1) * P, 0:SBK])
                xch0.append(t)
            for fc in range(NFC):
                nc.sync.dma_start(
                    wv_sb[:, fc * OLOC:(fc + 1) * OLOC], wv[fc * P:(fc + 1) * P, :])

            # ---- persistent activations (fp16 attention operands)
            qT = [[qkp.tile([P, SBK], F16, name=f"qT{ob}{sb}", tag=f"q{ob}{sb}")
                   for sb in range(NSB)] for ob in range(2)]
            kT = [[qkp.tile([P, SBK], F16, name=f"kT{ob}{sb}", tag=f"k{ob}{sb}")
                   for sb in range(NSB)] for ob in range(2)]
            # v natural per key-chunk: [128, 4*65]; head h at cols 65h..65h+63,
            # ones column at 65h+64 (softmax denominator lands in AV row 64).
            vt = [vtp.tile([P, HLOC * (HD + 1)], F16, name=f"vt{kc}", tag=f"v{kc}")
                  for kc in range(NKC)]
            ctxq = [[ctxp_pool.tile([P, SBK], F16, name=f"ctxq{ob}{qb}",
                                    tag=f"c{ob}{qb}") for qb in range(NSB)]
                    for ob in range(2)]

            masks_sb = constp.tile([P, 4 * SBK + 4], F16)
            nc.sync.dma_start(masks_sb[:], masks[:])
            bo_sb = constp.tile([P, 2], F32)
            nc.sync.dma_start(bo_sb[:], bo[:])
            ones_l = constp.tile([1, HD], F32R)
            nc.sync.dma_start(ones_l[:], ones_in[:])
            for kc in range(NKC):
                v3 = vt[kc].rearrange("p (h x) -> p h x", h=HLOC)
                nc.gpsimd.dma_start(v3[:, :, HD:HD + 1], masks_sb[:, 4 * SBK:])
            for fc in range(NFC):
                nc.sync.dma_start(
                    wo_sb[:, fc * OLOC:(fc + 1) * OLOC], wo[fc * P:(fc + 1) * P, :])

            cin_q = [dramp.tile([OLOC, SBK], F16, name=f"cin{qb}", tag=f"cin{qb}")
                     for qb in range(NSB)]
            cout_q = [dramp.tile([HLOC * OLOC, SBK], F16, name=f"cout{qb}",
                                 tag=f"cout{qb}") for qb in range(NSB)]

            # ======== one fused pass per seq-block: proj -> attn -> AG;
            # the out-projection of chunk qb is emitted one block later so the
            # PE never waits on the AllGather.
            pending_outproj = []
            for sb in range(NSB):
                # ---- q/k/v projections for this seq block
                if sb == 0:
                    xch = xch0
                else:
                    xch = []
                    for fc in range(NFC):
                        t = xcp.tile([P, SBK], F16, tag="x", name=f"x{sb}{fc}")
                        nc.sync.dma_start(
                            t[:], xt[fc * P:(fc + 1) * P, sb * SBK:(sb + 1) * SBK])
                        xch.append(t)
                for (w_sb, dst) in ((wq_sb, qT), (wk_sb, kT)):
                    for ob in range(2):
                        ps = psa.tile([P, SBK], F32, tag="acc", name=f"pq{sb}{ob}")
                        for fc in range(NFC):
                            lhsT = w_sb[:, fc * OLOC + ob * P:
                                        fc * OLOC + (ob + 1) * P]
                            nc.tensor.matmul(ps[:], lhsT, xch[fc][:],
                                             start=(fc == 0), stop=(fc == NFC - 1))
                        nc.vector.tensor_copy(dst[ob][sb][:], ps[:])
                for sc in range(4):
                    kc = 4 * sb + sc
                    ps = psa.tile([P, OLOC], F32, tag="acc", name=f"pv{sb}{sc}")
                    for fc in range(NFC):
                        lhsT = xch[fc][:, sc * P:(sc + 1) * P]
                        nc.tensor.matmul(ps[:], lhsT,
                                         wv_sb[:, fc * OLOC:(fc + 1) * OLOC],
                                         start=(fc == 0), stop=(fc == NFC - 1))
                    v3 = vt[kc].rearrange("p (h x) -> p h x", h=HLOC)
                    p3 = ps.rearrange("p (h x) -> p h x", h=HLOC)
                    nc.vector.tensor_copy(v3[:, :, 0:HD], p3[:])

                # ---- causal attention for q-chunk qb == sb
                qb = sb
                nkb = 4 * qb + 4
                # head h's denominator parked at partition 32h (alignment)
                den = smp.tile([P, SBK], F32, name=f"den{qb}", tag="den")
                rec = smp.tile([P, SBK], F32, name=f"rec{qb}", tag="rec")
                ctx_sb_h = []
                for ob in range(2):
                    for j in range(2):
                        h = 2 * ob + j
                        ctx_ps = psC.tile([HD + 1, SBK], F32, tag="ctx")
                        for kb in range(nkb):
                            t = kb - 4 * qb
                            off = 128 * t if t > 0 else 0
                            sp = psa.tile([P, SBK], F32, tag="acc",
                                          name=f"sp{qb}{h}{kb}")
                            nc.tensor.matmul(
                                sp[:, off:],
                                kT[ob][kb // 4][j * HD:(j + 1) * HD,
                                                (kb % 4) * P:(kb % 4 + 1) * P],
                                qT[ob][qb][j * HD:(j + 1) * HD, off:],
                                start=True, stop=True,
                                tile_position=(j * HD, 0),
                            )
                            et = expp.tile([P, SBK], F16, tag="e",
                                           name=f"et{qb}{h}{kb}")
                            nc.scalar.activation(et[:, off:], sp[:, off:],
                                                 Exp, scale=0.125)
                            if t >= 0:  # diagonal block: zero out k > q
                                nc.vector.tensor_tensor(
                                    et[:, off:], et[:, off:],
                                    masks_sb[:, t * SBK + off:(t + 1) * SBK],
                                    mybir.AluOpType.mult,
                                )
                            nc.tensor.matmul(
                                ctx_ps[:, off:],
                                vt[kb][:, h * (HD + 1):(h + 1) * (HD + 1)],
                                et[:, off:],
                                start=(kb == 0), stop=(kb == nkb - 1),
                            )
                        nc.vector.tensor_copy(den[32 * h:32 * h + 1, :],
                                              ctx_ps[HD:HD + 1, :])
                        cu = smp.tile([HD, SBK], F32, name=f"cu{qb}{h}",
                                      tag=f"cu{h}")
                        nc.scalar.copy(cu[:], ctx_ps[0:HD, :])
                        ctx_sb_h.append(cu)
                # batched reciprocal for the 4 heads of this q-chunk
                nc.vector.reciprocal(rec[:], den[:])
                for ob in range(2):
                    for j in range(2):
                        h = 2 * ob + j
                        rcr = smp.tile([1, SBK], F32R, tag="rcr",
                                       name=f"rcr{qb}{h}")
                        nc.vector.tensor_copy(rcr[:], rec[32 * h:32 * h + 1, :])
                        bc_ps = psB.tile([HD, SBK], F32, tag="bcp",
                                         name=f"bcp{qb}{h}")
                        nc.tensor.matmul(bc_ps[:], ones_l[:], rcr[:],
                                         start=True, stop=True)
                        bc = smp.tile([HD, SBK], F32, tag="bc", name=f"bc{qb}{h}")
                        nc.vector.tensor_copy(bc[:], bc_ps[:])
                        nc.vector.tensor_tensor(
                            ctxq[ob][qb][j * HD:(j + 1) * HD, :],
                            ctx_sb_h[h][:], bc[:],
                            mybir.AluOpType.mult,
                        )
                # ---- AllGather this q-chunk across the 4 cores of the batch
                nc.gpsimd.dma_start(cin_q[qb][0:P, :], ctxq[0][qb][:])
                nc.gpsimd.dma_start(cin_q[qb][P:OLOC, :], ctxq[1][qb][:])
                nc.gpsimd.collective_compute(
                    "AllGather",
                    mybir.AluOpType.bypass,
                    replica_groups=[[0, 1, 2, 3], [4, 5, 6, 7]],
                    ins=[cin_q[qb].opt()],
                    outs=[cout_q[qb].opt()],
                )
                def _emit_outproj(qb=qb):
                    g = [gthp.tile([P, SBK], F16, name=f"g{qb}{oc}",
                                   tag=f"g{qb % 2}{oc}") for oc in range(NFC)]
                    for oc in range(NFC):
                        nc.gpsimd.dma_start(g[oc][:],
                                            cout_q[qb][oc * P:(oc + 1) * P, :])
                    for cb in range(2):
                        ps = psa.tile([P, SBK], F32, tag="acc",
                                      name=f"po{qb}{cb}")
                        for oc in range(NFC):
                            lhsT = wo_sb[:, oc * OLOC + cb * P:
                                         oc * OLOC + (cb + 1) * P]
                            nc.tensor.matmul(ps[:], lhsT, g[oc][:],
                                             start=(oc == 0),
                                             stop=(oc == NFC - 1))
                        ot = osbp.tile([P, SBK], F32, tag="ot",
                                       name=f"ot{qb}{cb}")
                        nc.scalar.activation(ot[:], ps[:], Ident,
                                             bias=bo_sb[:, cb:cb + 1], scale=1.0)
                        nc.sync.dma_start(
                            outT[cb * P:(cb + 1) * P,
                                 qb * SBK:(qb + 1) * SBK], ot[:])
                pending_outproj.append(_emit_outproj)
                if sb >= 2:
                    pending_outproj.pop(0)()
            while pending_outproj:
                pending_outproj.pop(0)()

    _legalize_waits(nc)
    return nc


def _get_nc():
    if "nc" not in _CACHE:
        _CACHE["nc"] = _build_nc()
    return _CACHE["nc"]


LAST_RESULTS = None  # BassKernelResults of the most recent run (for profiling)


def kernel(x, Wq, Wk, Wv, Wo, bo):
    global LAST_RESULTS
    x = np.ascontiguousarray(np.asarray(x, dtype=np.float32))
    Wq = np.asarray(Wq, dtype=np.float32)
    Wk = np.asarray(Wk, dtype=np.float32)
    Wv = np.asarray(Wv, dtype=np.float32)
    Wo = np.asarray(Wo, dtype=np.float32)
    bo = np.asarray(bo, dtype=np.float32)

    # causal masks for the 4 diagonal 128x512 blocks (valid iff qi >= ki+128t)
    # plus 4 trailing ones-columns used to augment V.
    ki = np.arange(P)[:, None]
    qi = np.arange(SBK)[None, :]
    masks = np.concatenate(
        [(qi >= ki + P * t).astype(np.float16) for t in range(4)]
        + [np.ones((P, 4), dtype=np.float16)], axis=1)

    in_maps = []
    for c in range(NCORES):
        b, g = divmod(c, HLOC)
        sl = slice(g * OLOC, (g + 1) * OLOC)
        in_maps.append({
            "xt": np.ascontiguousarray(x[b].T.astype(np.float16)),
            "wq": np.ascontiguousarray(Wq[sl, :].T.astype(np.float16)),
            "wk": np.ascontiguousarray(Wk[sl, :].T.astype(np.float16)),
            "wv": np.ascontiguousarray(Wv[sl, :].T.astype(np.float16)),
            "wo": np.ascontiguousarray(Wo[sl, :].T.astype(np.float16)),
            "bo": np.ascontiguousarray(bo[sl].reshape(2, P).T),
            "masks": masks,
            "ones_in": np.ones((1, HD), dtype=np.float32),
        })

    nc = _get_nc()
    LAST_RESULTS = run_bass_kernel_spmd(nc, in_maps, core_ids=list(range(NCORES)))

    out = np.empty((B, S, D), dtype=np.float32)
    for c in range(NCORES):
        b, g = divmod(c, HLOC)
        out[b, :, g * OLOC:(g + 1) * OLOC] = LAST_RESULTS.results[c]["outT"].T
    return out
